# revision 3
# baseline (speedup 1.0000x reference)
"""Trainium2 Bass kernel for nn_Memory_10368051052717 (topk_masking).

Reference computation (per full problem):
  x: [32, 512, 32, 32] f32, mempool: [2000, 512] f32
  query = NCHW->NHWC flatten -> [32768, 512]
  att   = query @ mempool.T          [32768, 2000]
  val, idx = top_k(att, 10); val = softmax(val)
  attvec = scatter(val at idx)       [32768, 2000]
  out1  = attvec @ mempool  -> NCHW  [32, 512, 32, 32]
  out2  = attvec * att               [32768, 2000]
  returns (out1, out2)

Strategy: data-parallel over batch (4 batches per core x 8 cores).
x[b] is naturally [C=512, HW=1024] = Q^T, so matmul1 needs no input
transpose. Top-10 via DVE max8 -> match_replace -> max8 (threshold t =
10th largest), then attvec = (att >= t) * exp(att - rowmax + ln(inv))
in one fused scalar_tensor_tensor, out2 = attvec * att on GPSIMD.
attvec is PE-transposed per tile into a per-group collector so matmul2
(out1^T = mempool^T @ attvec^T) runs with N=512 moving columns.
"""

import functools
import sys

import numpy as np

for _p in ("/opt/trn_rl_repo",):
    if _p not in sys.path:
        sys.path.insert(0, _p)

B, C, HW = 32, 512, 1024  # batches, channels, H*W
M = 2000                  # mempool items
MP = 2048                 # padded items (zero rows)
K_TOP = 10
N_CORES = 8
NB = B // N_CORES         # batches per core = 4
TPB = HW // 128           # 128-row tiles per batch = 8
NEG = -1.0e30


def _build_program(nb=NB, tpb=TPB):
    import concourse.bass as bass
    import concourse.bacc as bacc
    import concourse.mybir as mybir
    import concourse.tile as tile

    f32 = mybir.dt.float32
    AF = mybir.ActivationFunctionType
    OP = mybir.AluOpType
    rows = 128 * tpb

    nc = bacc.Bacc("TRN2", target_bir_lowering=False, debug=False)

    xq_d = nc.dram_tensor("xq", [nb, C, rows], f32, kind="ExternalInput")
    mp_d = nc.dram_tensor("mp", [MP, C], f32, kind="ExternalInput")
    mpT_d = nc.dram_tensor("mpT", [C, MP], f32, kind="ExternalInput")
    id_d = nc.dram_tensor("ident", [128, 128], f32, kind="ExternalInput")
    out1_d = nc.dram_tensor("out1", [nb, C, rows], f32, kind="ExternalOutput")
    out2_d = nc.dram_tensor("out2", [nb * rows, M], f32, kind="ExternalOutput")

    KC = C // 128           # 4 k-chunks for matmul1
    MC = MP // 128          # 16 item chunks for matmul2
    GT = 4                  # tiles per matmul2 group
    n_groups = tpb // GT if tpb >= GT else 1
    gt = min(GT, tpb)

    with tile.TileContext(nc) as tc:
        with (
            tc.tile_pool(name="const", bufs=1) as cpool,
            tc.tile_pool(name="xq", bufs=1) as xqp,
            tc.tile_pool(name="attS", bufs=2) as attsp,
            tc.tile_pool(name="att2", bufs=1) as att2p,
            tc.tile_pool(name="E", bufs=2) as ep,
            tc.tile_pool(name="av", bufs=2) as avp,
            tc.tile_pool(name="o2", bufs=2) as o2p,
            tc.tile_pool(name="avT", bufs=1) as avtp,
            tc.tile_pool(name="o1", bufs=2) as o1p,
            tc.tile_pool(name="small", bufs=2) as sp,
            tc.tile_pool(name="attp", bufs=1, space="PSUM") as attpp,
            tc.tile_pool(name="tpp", bufs=2, space="PSUM") as tpp,
            tc.tile_pool(name="mm2p", bufs=2, space="PSUM") as mm2p,
        ):
            # ---- constants ----
            mpT_t = cpool.tile([128, KC, MP], f32)   # mempool^T  [c-part, kc, m]
            nc.sync.dma_start(
                mpT_t[:], mpT_d[:].rearrange("(a p) f -> p a f", p=128))
            mp_t = cpool.tile([128, MC, C], f32)     # mempool    [m-part, mc, c]
            nc.sync.dma_start(
                mp_t[:], mp_d[:].rearrange("(a p) f -> p a f", p=128))
            id_t = cpool.tile([128, 128], f32)
            nc.sync.dma_start(id_t[:], id_d[:])

            for b in range(nb):
                xq_t = xqp.tile([128, KC, rows], f32)
                nc.sync.dma_start(
                    xq_t[:], xq_d[b].rearrange("(a p) f -> p a f", p=128))

                for g in range(n_groups):
                    avT_t = avtp.tile([128, MC, gt * 128], f32)

                    for ti in range(gt):
                        t = g * gt + ti
                        r0 = 128 * t
                        # ---- matmul1: att[128 rows, MP] ----
                        att = attpp.tile([128, MP], f32)
                        for kc in range(KC):
                            lhsT = xq_t[:, kc, r0:r0 + 128]
                            for n in range(MP // 512):
                                nc.tensor.matmul(
                                    att[:, n * 512:(n + 1) * 512],
                                    lhsT,
                                    mpT_t[:, kc, n * 512:(n + 1) * 512],
                                    start=(kc == 0),
                                    stop=(kc == KC - 1),
                                )
                        # ---- att -> SBUF; pad cols forced to -inf ----
                        attS = attsp.tile([128, MP], f32)
                        nc.scalar.copy(attS[:], att[:])
                        nc.vector.memset(attS[:, M:MP], NEG)
                        # ---- top-10 threshold ----
                        v8a = sp.tile([128, 8], f32, tag="v8a")
                        nc.vector.max(out=v8a[:], in_=attS[:])
                        att2 = att2p.tile([128, MP], f32)
                        nc.vector.match_replace(
                            out=att2[:], in_to_replace=v8a[:],
                            in_values=attS[:], imm_value=NEG)
                        v8b = sp.tile([128, 8], f32, tag="v8b")
                        nc.vector.max(out=v8b[:], in_=att2[:])
                        # ---- softmax scalars ----
                        nm = sp.tile([128, 1], f32, tag="nm")       # -rowmax
                        nc.vector.tensor_scalar_mul(nm[:], v8a[:, 0:1], -1.0)
                        e8 = sp.tile([128, 8], f32, tag="e8")
                        s1 = sp.tile([128, 1], f32, tag="s1")
                        nc.scalar.activation(
                            e8[:], v8a[:], AF.Exp, bias=nm[:], scale=1.0,
                            accum_out=s1[:])
                        e2 = sp.tile([128, 2], f32, tag="e2")
                        s2 = sp.tile([128, 1], f32, tag="s2")
                        nc.scalar.activation(
                            e2[:], v8b[:, 0:2], AF.Exp, bias=nm[:], scale=1.0,
                            accum_out=s2[:])
                        den = sp.tile([128, 1], f32, tag="den")
                        nc.vector.tensor_add(den[:], s1[:], s2[:])
                        inv = sp.tile([128, 1], f32, tag="inv")
                        nc.vector.reciprocal(inv[:], den[:])
                        lni = sp.tile([128, 1], f32, tag="lni")
                        nc.scalar.activation(lni[:], inv[:], AF.Ln)
                        bp = sp.tile([128, 1], f32, tag="bp")       # ln(inv)-rowmax
                        nc.vector.tensor_add(bp[:], lni[:], nm[:])
                        # ---- E = exp(att - rowmax + ln(inv)) = softmax-if-kept ----
                        E = ep.tile([128, MP], f32)
                        nc.scalar.activation(E[:], attS[:], AF.Exp, bias=bp[:],
                                             scale=1.0)
                        # ---- attvec = (att >= t) * E ----
                        av = avp.tile([128, MP], f32)
                        nc.vector.scalar_tensor_tensor(
                            av[:], attS[:], v8b[:, 1:2], E[:],
                            op0=OP.is_ge, op1=OP.mult)
                        # ---- out2 = attvec * att (GPSIMD) ----
                        o2 = o2p.tile([128, M], f32)
                        nc.gpsimd.tensor_tensor(
                            o2[:], av[:, 0:M], attS[:, 0:M], OP.mult)
                        nc.sync.dma_start(
                            out2_d[b * rows + r0: b * rows + r0 + 128, :],
                            o2[:])
                        # ---- transpose attvec into group collector ----
                        for mq in range(MC // 4):
                            tp = tpp.tile([128, 512], f32)
                            for j in range(4):
                                m = 4 * mq + j
                                nc.tensor.transpose(
                                    tp[:, 128 * j:128 * (j + 1)],
                                    av[:, 128 * m:128 * (m + 1)], id_t[:])
                            nc.scalar.copy(
                                avT_t[:, 4 * mq:4 * mq + 4,
                                      ti * 128:(ti + 1) * 128],
                                tp[:].rearrange("p (a f) -> p a f", a=4))

                    # ---- matmul2: out1T[gchunk] = mp^T-chunks @ avT ----
                    for mc in range(KC):  # 4 chunks of C
                        acc = mm2p.tile([128, gt * 128], f32)
                        for kc in range(MC):
                            nc.tensor.matmul(
                                acc[:],
                                mp_t[:, kc, mc * 128:(mc + 1) * 128],
                                avT_t[:, kc, :],
                                start=(kc == 0),
                                stop=(kc == MC - 1),
                            )
                        o1 = o1p.tile([128, gt * 128], f32)
                        nc.vector.tensor_copy(o1[:], acc[:])
                        nc.sync.dma_start(
                            out1_d[b, mc * 128:(mc + 1) * 128,
                                   g * gt * 128:(g + 1) * gt * 128],
                            o1[:])

    nc.compile()
    return nc


@functools.lru_cache(maxsize=None)
def _get_program():
    return _build_program()


def _prep_consts(mempool):
    mp_nat = np.zeros((MP, C), np.float32)
    mp_nat[:M] = mempool
    mpT = np.zeros((C, MP), np.float32)
    mpT[:, :M] = mempool.T
    ident = np.eye(128, dtype=np.float32)
    return mp_nat, mpT, ident


def kernel(x, mempool):
    from concourse import bass_utils

    x = np.ascontiguousarray(x, dtype=np.float32)
    mempool = np.ascontiguousarray(mempool, dtype=np.float32)
    xr = x.reshape(B, C, HW)
    mp_nat, mpT, ident = _prep_consts(mempool)

    in_maps = []
    for c in range(N_CORES):
        in_maps.append({
            "xq": np.ascontiguousarray(xr[c * NB:(c + 1) * NB]),
            "mp": mp_nat, "mpT": mpT, "ident": ident,
        })

    nc = _get_program()
    res = bass_utils.run_bass_kernel_spmd(nc, in_maps, list(range(N_CORES)))
    outs = res.results

    out1 = np.concatenate([r["out1"] for r in outs], axis=0)  # [32, C, HW]
    out1 = out1.reshape(B, C, 32, 32)
    out2 = np.concatenate([r["out2"] for r in outs], axis=0)  # [32768, M]
    return out1, out2


# revision 33
# speedup vs baseline: 2819.8157x; 2819.8157x over previous
"""Trainium2 Bass kernel for nn_Memory_10368051052717 (topk_masking).

Reference computation (per full problem):
  x: [32, 512, 32, 32] f32, mempool: [2000, 512] f32
  query = NCHW->NHWC flatten -> [32768, 512]
  att   = query @ mempool.T          [32768, 2000]
  val, idx = top_k(att, 10); val = softmax(val)
  attvec = scatter(val at idx)       [32768, 2000]
  out1  = attvec @ mempool  -> NCHW  [32, 512, 32, 32]
  out2  = attvec * att               [32768, 2000]
  returns (out1, out2)

Sharding: data-parallel over batch (4 batches/core x 8 cores). x[b] is
naturally [C=512, HW=1024] = Q^T so matmul1 needs no input transpose,
and out1 is produced transposed ([C, HW]) which is exactly the NCHW
output layout.

Per 128-row tile:
  PE    matmul1 -> att [128,2000] in four PSUM quarter-banks.
        Precision: top-k selection demands ~f32 att (TF32/fp32r flips
        174/32768 rows -> 2.3e-2 rel err; bf16 far worse). Computed as
        a 3-pass fp16 hi/lo decomposition (q_hi*m_hi + q_hi*m_lo +
        q_lo*m_hi, f32 PSUM accumulate; dropped q_lo*m_lo ~2^-24 rel),
        3 cyc/row vs native fp32's 4 cyc/row.
  ACT   att -> SBUF (quarter copies), E2 = exp(att - rowmax)
  DVE   max8 -> match_replace -> max8: top-8 vals, 9th/10th, t = 10th
  Pool  attvec0 = (att >= t) * E2 (fused STT); out2 = (attvec0*inv)*att
  DVE   attvec = attvec0 * inv (in place, next pipeline step)
  PE    transpose attvec into 2-tile group collector (ACT copies cast
        to bf16); matmul2 in bf16 (no selection risk, out1-only ~4e-3)
        out1^T = mempool-chunk^T @ attvec^T, N=256

Only Exp/Copy activation functions are used (single table set -> one
LoadActFuncSet total). Emission is software-pipelined: transposes and
the in-place normalize of tile t-1 are emitted inside tile t's stage so
every engine stream interleaves two tiles.
"""

import functools
import sys

import numpy as np

for _p in ("/opt/trn_rl_repo",):
    if _p not in sys.path:
        sys.path.insert(0, _p)

B, C, HW = 32, 512, 1024  # batches, channels, H*W
M = 2000                  # mempool items
MP = 2048                 # padded items (zero rows)
K_TOP = 10
N_CORES = 8
NB = B // N_CORES         # batches per core = 4
TPB = HW // 128           # 128-row tiles per batch = 8
NEG = -1.0e30
GT = 2                    # tiles per matmul2 group


def _build_program(nb=NB, tpb=TPB):
    import concourse.bass as bass
    import concourse.bacc as bacc
    import concourse.mybir as mybir
    import concourse.tile as tile

    f32 = mybir.dt.float32
    bf16 = mybir.dt.bfloat16
    f16 = mybir.dt.float16
    AF = mybir.ActivationFunctionType
    OP = mybir.AluOpType
    rows = 128 * tpb

    nc = bacc.Bacc("TRN2", target_bir_lowering=False, debug=False)

    xq_d = nc.dram_tensor("xq", [nb, C, rows], f32, kind="ExternalInput")
    mp_d = nc.dram_tensor("mp", [MP, C], f16, kind="ExternalInput")
    mpTh_d = nc.dram_tensor("mpTh", [C, MP], f16, kind="ExternalInput")
    mpTl_d = nc.dram_tensor("mpTl", [C, MP], f16, kind="ExternalInput")
    id_d = nc.dram_tensor("ident", [128, 128], f16, kind="ExternalInput")
    out1_d = nc.dram_tensor("out1", [nb, C, rows], f32, kind="ExternalOutput")
    out2_d = nc.dram_tensor("out2", [nb * rows, M], f32, kind="ExternalOutput")

    KC = C // 128           # 4 k-chunks for matmul1 / c-chunks for matmul2
    MC = MP // 128          # 16 item chunks
    gt = min(GT, tpb)
    GR = gt * 128           # rows per matmul2 group

    with tile.TileContext(nc) as tc:
        with (
            tc.tile_pool(name="const", bufs=1) as cpool,
            tc.tile_pool(name="xq", bufs=2) as xqp,
            tc.tile_pool(name="attS", bufs=3) as attsp,
            tc.tile_pool(name="att2", bufs=1) as att2p,
            tc.tile_pool(name="E", bufs=2) as ep,
            tc.tile_pool(name="av", bufs=2) as avp,
            tc.tile_pool(name="o2", bufs=2) as o2p,
            tc.tile_pool(name="avT", bufs=2) as avtp,
            tc.tile_pool(name="o1", bufs=2) as o1p,
            tc.tile_pool(name="small", bufs=3) as sp,
            tc.tile_pool(name="attq", bufs=4, space="PSUM") as attqp,
            tc.tile_pool(name="pemix", bufs=2, space="PSUM") as pemix,
        ):
            # ---- constants (mpT first: matmul1 needs it; mp only at mm2) ----
            mpTh_t = cpool.tile([128, KC, MP], f16)  # mempool^T hi [c, kc, m]
            nc.sync.dma_start(
                mpTh_t[:], mpTh_d[:].rearrange("(a p) f -> p a f", p=128))
            mpTl_t = cpool.tile([128, KC, MP], f16)  # mempool^T lo
            nc.sync.dma_start(
                mpTl_t[:], mpTl_d[:].rearrange("(a p) f -> p a f", p=128))
            id_t = cpool.tile([128, 128], f16)
            nc.sync.dma_start(id_t[:], id_d[:])
            mp_t = cpool.tile([128, MC, C], f16)    # mempool    [m-part, mc, c]
            nc.sync.dma_start(
                mp_t[:], mp_d[:].rearrange("(a p) f -> p a f", p=128))

            xq_tiles = {}

            def emit_xq_load(b, g):
                """Load group g's rows of batch b and split into f16 hi/lo."""
                xq_t = xqp.tile([128, KC, GR], f32)
                nc.sync.dma_start(
                    xq_t[:],
                    xq_d[b, :, g * GR:(g + 1) * GR].rearrange(
                        "(a p) f -> p a f", p=128))
                qh = xqp.tile([128, KC, GR], f16, tag="qh")
                nc.scalar.copy(qh[:], xq_t[:])          # f32 -> f16 round
                ql = xqp.tile([128, KC, GR], f16, tag="ql")
                nc.vector.tensor_sub(ql[:], xq_t[:], qh[:])  # residual -> f16
                xq_tiles[(b, g)] = (qh, ql)

            def emit_mm1(b, t):
                """matmul1 -> att in four PSUM quarters (q completes early).

                3-pass fp16 hi/lo pseudo-fp32: hi*hi + hi*lo + lo*hi,
                accumulated in f32 PSUM."""
                g, ti = divmod(t, gt)
                qh, ql = xq_tiles[(b, g)]
                r0 = 128 * ti
                quarters = []
                for q in range(4):
                    w = 512 if q < 3 else M - 3 * 512   # 2000 = 3*512 + 464
                    att_q = attqp.tile([128, 512], f32, tag="attq")
                    passes = [(qh, mpTh_t), (qh, mpTl_t), (ql, mpTh_t)]
                    np_ = len(passes)
                    for pi, (qa, mb_) in enumerate(passes):
                        for kc in range(KC):
                            nc.tensor.matmul(
                                att_q[:, 0:w],
                                qa[:, kc, r0:r0 + 128],
                                mb_[:, kc, q * 512:q * 512 + w],
                                start=(pi == 0 and kc == 0),
                                stop=(pi == np_ - 1 and kc == KC - 1),
                            )
                    quarters.append((att_q, w))
                return quarters

            def emit_finish(ctx):
                """Normalize attvec of tile t-1, form out2, transpose it."""
                (b, t, attS, av, inv, avT_t, ti) = ctx
                rows = 128 * tpb
                r0 = 128 * t
                nc.vector.tensor_scalar_mul(av[:], av[:], inv[:])
                o2 = o2p.tile([128, M], f32)
                nc.gpsimd.tensor_tensor(
                    o2[:], av[:, 0:M], attS[:, 0:M], OP.mult)
                nc.sync.dma_start(
                    out2_d[b * rows + r0: b * rows + r0 + 128, :], o2[:])
                for mq in range(MC // 4):
                    tp = pemix.tile([128, 512], f16, tag="tp16")
                    for j in range(4):
                        m = 4 * mq + j
                        nc.tensor.transpose(
                            tp[:, 128 * j:128 * (j + 1)],
                            av[:, 128 * m:128 * (m + 1)],
                            id_t[:])
                    nc.scalar.copy(
                        avT_t[:, 4 * mq:4 * mq + 4, ti * 128:(ti + 1) * 128],
                        tp[:].rearrange("p (a f) -> p a f", a=4))

            def emit_mm2(b, g, avT_t):
                for mc in range(KC):
                    acc = pemix.tile([128, GR], f32, tag="pemix")
                    for kc in range(MC):
                        nc.tensor.matmul(
                            acc[:],
                            mp_t[:, kc, mc * 128:(mc + 1) * 128],
                            avT_t[:, kc, :],
                            start=(kc == 0),
                            stop=(kc == MC - 1),
                        )
                    o1 = o1p.tile([128, GR], f32)
                    nc.scalar.copy(o1[:], acc[:])
                    nc.sync.dma_start(
                        out1_d[b, mc * 128:(mc + 1) * 128,
                               g * GR:(g + 1) * GR],
                        o1[:])

            def emit_stage(b, t, quarters, avT_t, prev_ctx):
                """Post-matmul1 work for tile t; finishes tile t-1 inline."""
                g, ti = divmod(t, gt)
                r0 = 128 * t
                # att -> SBUF (quarter copies on ACT), pad = -inf (Pool)
                attS = attsp.tile([128, MP], f32)
                for q, (att_q, w) in enumerate(quarters):
                    nc.scalar.copy(attS[:, q * 512:q * 512 + w],
                                   att_q[:, 0:w])
                nc.gpsimd.memset(attS[:, M:MP], NEG)
                # top-8
                v8a = sp.tile([128, 8], f32, tag="v8a")
                nc.vector.max(out=v8a[:], in_=attS[:])
                nm = sp.tile([128, 1], f32, tag="nm")       # -rowmax
                nc.vector.tensor_scalar_mul(nm[:], v8a[:, 0:1], -1.0)
                # ---- previous tile's normalize + transposes slot in here ----
                if prev_ctx is not None:
                    emit_finish(prev_ctx)
                    (pb, pt, _, _, _, pav, pti) = prev_ctx
                    if pti == gt - 1:
                        emit_mm2(pb, pt // gt, pav)
                # early ACT: exp sums of top-8, E2 = exp(att - rowmax)
                e8 = sp.tile([128, 8], f32, tag="e8")
                s1 = sp.tile([128, 1], f32, tag="s1")
                nc.scalar.activation(
                    e8[:], v8a[:], AF.Exp, bias=nm[:], scale=1.0,
                    accum_out=s1[:])
                E2 = ep.tile([128, MP], f32)
                nc.scalar.activation(E2[:], attS[:], AF.Exp, bias=nm[:],
                                     scale=1.0)
                # 9th/10th via match_replace + second max
                att2 = att2p.tile([128, MP], f32)
                nc.vector.match_replace(
                    out=att2[:], in_to_replace=v8a[:],
                    in_values=attS[:], imm_value=NEG)
                v8b = sp.tile([128, 8], f32, tag="v8b")
                nc.vector.max(out=v8b[:], in_=att2[:])
                e2 = sp.tile([128, 2], f32, tag="e2")
                s2 = sp.tile([128, 1], f32, tag="s2")
                nc.scalar.activation(
                    e2[:], v8b[:, 0:2], AF.Exp, bias=nm[:], scale=1.0,
                    accum_out=s2[:])
                den = sp.tile([128, 1], f32, tag="den")
                nc.vector.tensor_add(den[:], s1[:], s2[:])
                inv = sp.tile([128, 1], f32, tag="inv")
                nc.vector.reciprocal(inv[:], den[:])
                # attvec0 = (att >= t) * E2   (DVE fused STT)
                av = avp.tile([128, MP], f16)
                nc.vector.scalar_tensor_tensor(
                    av[:], attS[:], v8b[:, 1:2], E2[:],
                    op0=OP.is_ge, op1=OP.mult)
                return (b, t, attS, av, inv, avT_t, ti)

            # ---- pipelined emission ----
            tiles = [(b, t) for b in range(nb) for t in range(tpb)]
            avT_cur = None
            prev_ctx = None
            for b, t in tiles:
                g, ti = divmod(t, gt)
                if ti == 0:
                    emit_xq_load(b, g)
                    avT_cur = avtp.tile([128, MC, GR], f16)
                quarters = emit_mm1(b, t)
                prev_ctx = emit_stage(b, t, quarters, avT_cur, prev_ctx)
            emit_finish(prev_ctx)
            (pb, pt, _, _, _, pav, pti) = prev_ctx
            if pti == gt - 1:
                emit_mm2(pb, pt // gt, pav)

    nc.compile()
    return nc


@functools.lru_cache(maxsize=None)
def _get_program():
    return _build_program()


def _prep_consts(mempool):
    import ml_dtypes
    mp_nat = np.zeros((MP, C), np.float16)
    mp_nat[:M] = mempool.astype(np.float16)
    mpT = np.zeros((C, MP), np.float32)
    mpT[:, :M] = mempool.T
    mpTh = mpT.astype(np.float16)
    mpTl = (mpT - mpTh.astype(np.float32)).astype(np.float16)
    ident = np.eye(128, dtype=np.float16)
    return mp_nat, mpTh, mpTl, ident


def kernel(x, mempool):
    from concourse import bass_utils

    x = np.ascontiguousarray(x, dtype=np.float32)
    mempool = np.ascontiguousarray(mempool, dtype=np.float32)
    xr = x.reshape(B, C, HW)
    mp_nat, mpTh, mpTl, ident = _prep_consts(mempool)

    in_maps = []
    for c in range(N_CORES):
        in_maps.append({
            "xq": np.ascontiguousarray(xr[c * NB:(c + 1) * NB]),
            "mp": mp_nat, "mpTh": mpTh, "mpTl": mpTl, "ident": ident,
        })

    nc = _get_program()
    res = bass_utils.run_bass_kernel_spmd(nc, in_maps, list(range(N_CORES)))
    outs = res.results

    out1 = np.concatenate([r["out1"] for r in outs], axis=0)  # [32, C, HW]
    out1 = out1.reshape(B, C, 32, 32)
    out2 = np.concatenate([r["out2"] for r in outs], axis=0)  # [32768, M]
    return out1, out2
